# revision 9
# baseline (speedup 1.0000x reference)
"""Trainium2 Bass kernel for nn_EdgeFeatureInit (DimeNet-style edge feature init).

Computes, for E=320000 edges over N=20000 nodes with H=128, R=16:
    rbf  = norm * sin(d * freq) / d                     [E, R]
    rbf0 = silu(rbf @ W_rbf0 + b_rbf0)                  [E, H]
    e1   = silu([src|dst|rbf0] @ W_lin + b_lin)         [E, H]
    e2   = (rbf @ W_rbf1) * e1                          [E, H]

Strategy (8 NeuronCores, SPMD, no cross-core comms):
  - Edges sharded 40000/core, padded to 40960 = 20 macro-tiles x 2048.
  - Everything on chip lives in "layout B": hidden dim on the 128 SBUF
    partitions, edges on the free dim. Node gathers use
    gpsimd.dma_gather(transpose=True) from a bf16 copy of node_embs, which
    lands gathered rows directly as columns -> no on-chip transposes.
  - The three K=128 chunks of W_lin accumulate into one PSUM tile.
  - Bessel RBF: arg = d*freq + pi (DVE, per-partition freq scalar),
    arg mod 2pi (GPSIMD), ACT Sin with bias -pi (table domain is [-pi, pi]),
    then one DVE multiply by 1/d (host-precomputed; norm is folded into the
    rbf weight matrices host-side).
  - Outputs are produced transposed [H, E] per core and un-transposed on host.
"""

import os
import sys

import numpy as np

sys.path.insert(0, "/opt/trn_rl_repo")

import ml_dtypes

from concourse import bacc, bass, mybir
import concourse.tile as tile
from concourse.bass_utils import run_bass_kernel_spmd

BF16 = ml_dtypes.bfloat16

N_CORES = 8
N_NODES = 20000
N_EDGES = 320000
R = 16
H = 128
CUTOFF = 5.0

E_CORE = N_EDGES // N_CORES          # 40000
TE = 2048                            # macro-tile edges
E_PAD = 40960                        # E_CORE padded to TE multiple
N_MACRO = E_PAD // TE                # 20
MM = 512                             # matmul moving free dim

PI = float(np.pi)
TWO_PI = float(2 * np.pi)

_CACHE = {}
LAST_RESULT = None


def _build():
    """Build and compile the per-core Bass program (identical on all cores)."""
    f32 = mybir.dt.float32
    bf16 = mybir.dt.bfloat16
    i16 = mybir.dt.int16

    nc = bacc.Bacc("TRN2", target_bir_lowering=False, debug=False)

    i32 = mybir.dt.int32

    node_b = nc.dram_tensor("node_b", [N_NODES, H], bf16, kind="ExternalInput")
    wl = nc.dram_tensor("wl", [3 * H, H], bf16, kind="ExternalInput")
    w01 = nc.dram_tensor("w01", [R, 2 * H], bf16, kind="ExternalInput")
    fr16 = nc.dram_tensor("fr16", [R, 1], f32, kind="ExternalInput")  # freq/(2*pi)
    b0c = nc.dram_tensor("b0c", [H, 1], f32, kind="ExternalInput")
    blc = nc.dram_tensor("blc", [H, 1], f32, kind="ExternalInput")
    d16 = nc.dram_tensor("d16", [R, E_PAD], f32, kind="ExternalInput")
    r16 = nc.dram_tensor("r16", [R, E_PAD], f32, kind="ExternalInput")
    isrc = nc.dram_tensor("isrc", [128, E_PAD // 16], i16, kind="ExternalInput")
    idst = nc.dram_tensor("idst", [128, E_PAD // 16], i16, kind="ExternalInput")

    e1t = nc.dram_tensor("e1t", [H, E_PAD], f32, kind="ExternalOutput")
    e2t = nc.dram_tensor("e2t", [H, E_PAD], f32, kind="ExternalOutput")

    with tile.TileContext(nc) as tc:
        with (
            tc.tile_pool(name="const", bufs=1) as cpool,
            tc.tile_pool(name="work", bufs=2) as wpool,
            tc.tile_pool(name="psum", bufs=1, space="PSUM") as ppool,
        ):
            # ---- constants, loaded once ----
            wl_sb = cpool.tile([128, 3, H], bf16)
            for c in range(3):
                nc.sync.dma_start(out=wl_sb[:, c, :], in_=wl[c * 128:(c + 1) * 128, :])
            w01_sb = cpool.tile([R, 2 * H], bf16)
            nc.sync.dma_start(out=w01_sb[:], in_=w01[:])
            fr_sb = cpool.tile([R, 1], f32)
            nc.sync.dma_start(out=fr_sb[:], in_=fr16[:])
            b0_sb = cpool.tile([H, 1], f32)
            nc.sync.dma_start(out=b0_sb[:], in_=b0c[:])
            bl_sb = cpool.tile([H, 1], f32)
            nc.sync.dma_start(out=bl_sb[:], in_=blc[:])
            isrc_sb = cpool.tile([128, E_PAD // 16], i16)
            nc.sync.dma_start(out=isrc_sb[:], in_=isrc[:])
            idst_sb = cpool.tile([128, E_PAD // 16], i16)
            nc.sync.dma_start(out=idst_sb[:], in_=idst[:])
            zero_sb = cpool.tile([R, 1], f32)
            nc.vector.memset(zero_sb[:], 0.0)

            for t in range(N_MACRO):
                e0 = t * TE
                esl = slice(e0, e0 + TE)

                d_sb = wpool.tile([R, TE], f32)
                nc.sync.dma_start(out=d_sb[:], in_=d16[:, esl])
                r_sb = wpool.tile([R, TE], f32)
                nc.sync.dma_start(out=r_sb[:], in_=r16[:, esl])

                # gathers: node rows land as columns (transposed) in bf16
                srcT = wpool.tile([128, 1, TE], bf16)
                nc.gpsimd.dma_gather(
                    srcT[:], node_b[:, :], isrc_sb[:, t * (TE // 16):(t + 1) * (TE // 16)],
                    num_idxs=TE, num_idxs_reg=TE, elem_size=H,
                    transpose=True, single_packet=False,
                )
                dstT = wpool.tile([128, 1, TE], bf16)
                nc.gpsimd.dma_gather(
                    dstT[:], node_b[:, :], idst_sb[:, t * (TE // 16):(t + 1) * (TE // 16)],
                    num_idxs=TE, num_idxs_reg=TE, elem_size=H,
                    transpose=True, single_packet=False,
                )

                # ---- RBF front-end (16 partitions wide) ----
                # q = d*freq/(2pi); k = rne(q) (DVE int cast rounds-to-nearest-even);
                # u = q - k in [-0.5, 0.5]; sin(d*freq) = sin(2pi*u).
                ki_sb = wpool.tile([R, TE], i32)
                nc.vector.tensor_scalar(
                    out=ki_sb[:], in0=d_sb[:], scalar1=fr_sb[:, :1], scalar2=None,
                    op0=mybir.AluOpType.mult,
                )
                u_sb = wpool.tile([R, TE], f32)
                nc.vector.scalar_tensor_tensor(
                    out=u_sb[:], in0=d_sb[:], scalar=fr_sb[:, :1], in1=ki_sb[:],
                    op0=mybir.AluOpType.mult, op1=mybir.AluOpType.subtract,
                )
                sin_sb = wpool.tile([R, TE], f32)
                nc.scalar.activation(
                    sin_sb[:], u_sb[:], mybir.ActivationFunctionType.Sin,
                    bias=zero_sb[:, :1], scale=TWO_PI,
                )
                rbf_sb = wpool.tile([R, TE], bf16)
                nc.vector.tensor_tensor(
                    out=rbf_sb[:], in0=sin_sb[:], in1=r_sb[:],
                    op=mybir.AluOpType.mult,
                )

                # ---- matmuls ----
                # p_a: first rbf0 = rbf @ W_rbf0, then reused for e2pre = rbf @ W_rbf1
                p_a = ppool.tile([H, TE], f32, tag="p_a")
                p_e1 = ppool.tile([H, TE], f32, tag="p_e1")

                for k in range(TE // MM):
                    ms = slice(k * MM, (k + 1) * MM)
                    nc.tensor.matmul(
                        p_a[:, ms], lhsT=w01_sb[:, 0:H], rhs=rbf_sb[:, ms],
                        start=True, stop=True,
                    )
                rbf0_sb = wpool.tile([H, TE], bf16)
                nc.scalar.activation(
                    rbf0_sb[:], p_a[:], mybir.ActivationFunctionType.Silu,
                    bias=b0_sb[:, :1], scale=1.0,
                )

                for k in range(TE // MM):
                    ms = slice(k * MM, (k + 1) * MM)
                    nc.tensor.matmul(
                        p_e1[:, ms], lhsT=wl_sb[:, 0, :], rhs=srcT[:, 0, ms],
                        start=True, stop=False,
                    )
                for k in range(TE // MM):
                    ms = slice(k * MM, (k + 1) * MM)
                    nc.tensor.matmul(
                        p_e1[:, ms], lhsT=wl_sb[:, 1, :], rhs=dstT[:, 0, ms],
                        start=False, stop=False,
                    )
                for k in range(TE // MM):
                    ms = slice(k * MM, (k + 1) * MM)
                    nc.tensor.matmul(
                        p_e1[:, ms], lhsT=wl_sb[:, 2, :], rhs=rbf0_sb[:, ms],
                        start=False, stop=True,
                    )

                e1_sb = wpool.tile([H, TE], f32)
                nc.scalar.activation(
                    e1_sb[:], p_e1[:], mybir.ActivationFunctionType.Silu,
                    bias=bl_sb[:, :1], scale=1.0,
                )

                # e2pre reuses p_a after the rbf0 silu has read it
                for k in range(TE // MM):
                    ms = slice(k * MM, (k + 1) * MM)
                    nc.tensor.matmul(
                        p_a[:, ms], lhsT=w01_sb[:, H:2 * H], rhs=rbf_sb[:, ms],
                        start=True, stop=True,
                    )
                e2_sb = wpool.tile([H, TE], f32)
                nc.vector.tensor_tensor(
                    out=e2_sb[:], in0=p_a[:], in1=e1_sb[:],
                    op=mybir.AluOpType.mult,
                )

                nc.sync.dma_start(out=e1t[:, esl], in_=e1_sb[:])
                nc.sync.dma_start(out=e2t[:, esl], in_=e2_sb[:])

    nc.compile()
    return nc


def _prep_core_inputs(node_b, wl_b, w01_b, fr16, b0c, blc, ei, d):
    """Build the per-core input map from this core's edge slice (len E_CORE)."""
    dp = np.full(E_PAD, 1.0, dtype=np.float32)
    dp[:E_CORE] = d
    ip = np.zeros((2, E_PAD), dtype=np.int16)
    ip[:, :E_CORE] = ei.astype(np.int16)

    d16 = np.broadcast_to(dp[None, :], (R, E_PAD)).copy()
    r16 = np.broadcast_to((np.float32(1.0) / dp)[None, :], (R, E_PAD)).copy()

    # dma_gather index layout: idx i lives at [i % 16, i // 16], replicated
    # across the 8 groups of 16 partitions.
    def swizzle(ix):
        sw = ix.reshape(E_PAD // 16, 16).T.copy()           # [16, E_PAD//16]
        return np.tile(sw, (8, 1)).copy()                   # [128, E_PAD//16]

    return {
        "node_b": node_b, "wl": wl_b, "w01": w01_b, "fr16": fr16,
        "b0c": b0c, "blc": blc, "d16": d16, "r16": r16,
        "isrc": swizzle(ip[0]), "idst": swizzle(ip[1]),
    }


def kernel(node_embs, edge_index, edge_weight, W_rbf0, b_rbf0, W_lin, b_lin, W_rbf1):
    global LAST_RESULT
    node_embs = np.asarray(node_embs)
    edge_index = np.asarray(edge_index)
    edge_weight = np.asarray(edge_weight)

    norm = np.float32(np.sqrt(2.0 / CUTOFF))
    node_b = node_embs.astype(BF16)
    wl_b = np.asarray(W_lin, dtype=np.float32).astype(BF16)
    w01_b = np.concatenate(
        [norm * np.asarray(W_rbf0, np.float32), norm * np.asarray(W_rbf1, np.float32)],
        axis=1,
    ).astype(BF16)                                          # [R, 2H]
    fr16 = (np.arange(1, R + 1, dtype=np.float64) * (np.pi / CUTOFF) / (2 * np.pi)).astype(np.float32).reshape(R, 1)
    b0c = np.asarray(b_rbf0, np.float32).reshape(H, 1).copy()
    blc = np.asarray(b_lin, np.float32).reshape(H, 1).copy()

    if "nc" not in _CACHE:
        _CACHE["nc"] = _build()
    nc = _CACHE["nc"]

    in_maps = []
    for c in range(N_CORES):
        sl = slice(c * E_CORE, (c + 1) * E_CORE)
        in_maps.append(_prep_core_inputs(
            node_b, wl_b, w01_b, fr16, b0c, blc,
            edge_index[:, sl], edge_weight[sl],
        ))

    res = run_bass_kernel_spmd(nc, in_maps, core_ids=list(range(N_CORES)))
    LAST_RESULT = res

    e1 = np.empty((N_EDGES, H), dtype=np.float32)
    e2 = np.empty((N_EDGES, H), dtype=np.float32)
    for c in range(N_CORES):
        sl = slice(c * E_CORE, (c + 1) * E_CORE)
        e1[sl] = res.results[c]["e1t"][:, :E_CORE].T
        e2[sl] = res.results[c]["e2t"][:, :E_CORE].T
    return (e1, e2)
